# revision 83
# baseline (speedup 1.0000x reference)
"""Trainium2 Bass kernel for nn_MultiHeadRot (RoPE multi-head causal attention).

Sharding: tensor-parallel over heads — each of the 8 cores owns 2 of the 16
heads: it computes the QKV projection for its head pair, RoPE, causal
attention, and a partial output projection against its 128-column slice of
w_out. The host sums the 8 partial outputs (the TP all-reduce happens at
gather time).

Layout choices (per core):
  - Activations live feature-major on chip: xT/q/k/ctx are [d_model|128, tokens].
  - RoPE uses a "half-split" head_dim permutation (applied to the q/k weight
    rows on the host) so rotate_half becomes a fixed 128x128 matmul R.
  - Attention computes transposed scores S^T = K_blk^T Q_chunk ([k=128, q=512]
    per block, both heads running concurrently on 64-row PE quadrants), exp on
    ScalarE into bf16 probs, multiplicative staircase masks on the
    causal-diagonal blocks (GpSimd), and P V via a [k,65] stationary operand
    whose 65th column of ones accumulates the softmax denominators alongside
    the context.
  - Softmax normalization: reciprocal_approx_fast on the [1, 2*512] denominator
    rows, DMA partition-broadcast to [64, 2, 512], fused into the PSUM->SBUF
    context drain (TensorTensor mult).
  - Scheduling: the PE stream is software-pipelined. Scores(kj+1) is emitted
    between Scores(kj) and PV(kj) so the exp latency hides; projection work
    for chunk i+2 and output-projection work for chunk i-1 are paced into
    chunk i's attention stream weighted by estimated PE cost, keeping the PE
    dense (idle gaps also drop the PE p-state, costing ~2x).
All matmuls run in bf16 (fp32 PSUM accumulation); softmax runs in fp32.
"""

import sys

for _p in ("/opt/trn_rl_repo", "/opt/pypackages"):
    if _p not in sys.path:
        sys.path.insert(0, _p)

from collections import deque

import numpy as np
import ml_dtypes

BF16 = ml_dtypes.bfloat16

B, S, D, NH, HD = 4, 2048, 1024, 16, 64
T = B * S
NCORES = 8
CH = 512          # token chunk (free dim) for projections / attention q-chunks
NCHUNK = T // CH  # 16
NCI = S // CH     # 4 chunks per sequence

_PROGRAM = None


def _build_program():
    import concourse.bass as bass
    import concourse.mybir as mybir
    import concourse.tile as tile
    from concourse import bacc
    from concourse.bass import ds

    dt = mybir.dt
    AF = mybir.ActivationFunctionType

    nc = bacc.Bacc("TRN2", debug=False)

    xT_d = nc.dram_tensor("xT", [D, T], dt.bfloat16, kind="ExternalInput")
    wq_d = nc.dram_tensor("wqT", [128, 8 * 128], dt.bfloat16, kind="ExternalInput")
    wk_d = nc.dram_tensor("wkT", [128, 8 * 128], dt.bfloat16, kind="ExternalInput")
    wv_d = nc.dram_tensor("wvT", [128, 8 * 128], dt.bfloat16, kind="ExternalInput")
    wo_d = nc.dram_tensor("woT", [128, D], dt.bfloat16, kind="ExternalInput")
    rT_d = nc.dram_tensor("rT", [128, 128], dt.bfloat16, kind="ExternalInput")
    cos_d = nc.dram_tensor("cosT", [128, S], dt.bfloat16, kind="ExternalInput")
    sin_d = nc.dram_tensor("sinT", [128, S], dt.bfloat16, kind="ExternalInput")
    mask_d = nc.dram_tensor("masks", [128, 4 * CH], dt.bfloat16, kind="ExternalInput")
    out_d = nc.dram_tensor("out", [T, D], dt.bfloat16, kind="ExternalOutput")

    with tile.TileContext(nc) as tc:
        with (
            tc.tile_pool(name="const", bufs=1) as cp,
            tc.tile_pool(name="persist", bufs=1) as pp,
            tc.tile_pool(name="xs", bufs=6) as xp,
            tc.tile_pool(name="psA", bufs=2, space="PSUM") as psA,
            tc.tile_pool(name="spsum", bufs=2, space="PSUM") as sps,
            tc.tile_pool(name="ctxpsum", bufs=2, space="PSUM") as cps,
            tc.tile_pool(name="sb1", bufs=6) as sb1,
            tc.tile_pool(name="sb2", bufs=8) as sb2,
            tc.tile_pool(name="sbt", bufs=3) as sbt,
            tc.tile_pool(name="sb3", bufs=6) as sb3,
        ):
            wq_sb = cp.tile([128, 8, 128], dt.bfloat16, tag="wq")
            wk_sb = cp.tile([128, 8, 128], dt.bfloat16, tag="wk")
            wv_sb = cp.tile([128, 8, 128], dt.bfloat16, tag="wv")
            wo_sb = cp.tile([128, D], dt.bfloat16, tag="wo")
            rT_sb = cp.tile([128, 128], dt.bfloat16, tag="rt")
            cos_sb = cp.tile([128, S], dt.bfloat16, tag="cos")
            sin_sb = cp.tile([128, S], dt.bfloat16, tag="sin")
            mask_sb = cp.tile([128, 4, CH], dt.bfloat16, tag="mask")

            qrot = pp.tile([128, NCHUNK, CH], dt.bfloat16, tag="qrot")
            krot = pp.tile([128, NCHUNK, CH], dt.bfloat16, tag="krot")
            vsb = pp.tile([128, T // 128, 130], dt.bfloat16, tag="vsb")
            ctx = pp.tile([128, NCHUNK, CH], dt.bfloat16, tag="ctx")

            xT_ap = xT_d.ap().rearrange("(i p) t -> p i t", p=128)
            out_ap = out_d.ap().rearrange("(tb p) e -> tb p e", p=128)

            xt_tiles = {}

            def xt_dma(ch):
                xt = xp.tile([128, 8, CH], dt.bfloat16, tag="xt", name="xt")
                nc.sync.dma_start(xt[:], xT_ap[:, :, ds(ch * CH, CH)])
                xt_tiles[ch] = xt

            # ---- startup DMAs: the pieces the first projection item needs
            # (wq, xt0, rT, cos, sin) lead their queues; xt0 split across two
            # queues so the first chunk lands fast.
            nc.sync.dma_start(wq_sb[:], wq_d.ap().rearrange("p (i o) -> p i o", i=8))
            # chunk-0 x arrives as 8 independent tiles across three queues so
            # projection matmul i waits only on its own slice
            xt0_parts = []
            engs = (nc.scalar, nc.sync, nc.gpsimd)
            for i in range(8):
                t = cp.tile([128, CH], dt.bfloat16, tag=f"x0_{i}")
                engs[i % 3].dma_start(t[:], xT_ap[:, i, ds(0, CH)])
                xt0_parts.append(t)
            xt_tiles[0] = xt0_parts
            nc.gpsimd.dma_start(rT_sb[:], rT_d.ap())
            nc.gpsimd.dma_start(cos_sb[:], cos_d.ap())
            nc.scalar.dma_start(sin_sb[:], sin_d.ap())
            nc.sync.dma_start(wk_sb[:], wk_d.ap().rearrange("p (i o) -> p i o", i=8))
            nc.gpsimd.dma_start(wv_sb[:], wv_d.ap().rearrange("p (i o) -> p i o", i=8))
            nc.scalar.dma_start(wo_sb[:], wo_d.ap())
            nc.gpsimd.dma_start(
                mask_sb[:], mask_d.ap().rearrange("p (j q) -> p j q", j=4)
            )
            nc.vector.memset(vsb[:, :, 64:65], 1.0)
            nc.vector.memset(vsb[:, :, 129:130], 1.0)
            xt_dma(1)
            xt_dma(2)

            # ---- work item generators; each yields (est_pe_ns, emit_fn) ----

            def proj_items(ch):
                """QKV projection + RoPE for chunk ch (xt DMA'd earlier).
                xt lookup deferred to emission time — the DMA item that
                creates the tile may still be queued ahead of us."""
                s0 = (ch % NCI) * CH

                def xt_sl(i):
                    xt = xt_tiles[ch]
                    return xt[i][:] if isinstance(xt, list) else xt[:, i]

                def qk_item(w_sb, dst):
                    ps = psA.tile([128, CH], dt.float32, tag="a", name="projps")
                    for i in range(8):
                        nc.tensor.matmul(
                            ps[:], w_sb[:, i], xt_sl(i),
                            start=(i == 0), stop=(i == 7),
                        )
                    qk_bf = sb1.tile([128, CH], dt.bfloat16, tag="qkbf", name="qkbf")
                    nc.vector.tensor_copy(qk_bf[:], ps[:])
                    rot = psA.tile([128, CH], dt.float32, tag="a", name="rotps")
                    nc.tensor.matmul(rot[:], rT_sb[:], qk_bf[:], start=True, stop=True)
                    t1 = sb1.tile([128, CH], dt.bfloat16, tag="t1", name="t1")
                    t2 = sb1.tile([128, CH], dt.bfloat16, tag="t2", name="t2")
                    nc.vector.tensor_mul(t1[:], qk_bf[:], cos_sb[:, ds(s0, CH)])
                    nc.vector.tensor_mul(t2[:], rot[:], sin_sb[:, ds(s0, CH)])
                    nc.vector.tensor_add(dst[:, ch], t1[:], t2[:])

                def v_item(j):
                    tb = ch * 4 + j
                    pv = psA.tile([128, 128], dt.float32, tag="a", name="vps")
                    for i in range(8):
                        nc.tensor.matmul(
                            pv[:], xt_sl(i)[:, ds(j * 128, 128)], wv_sb[:, i],
                            start=(i == 0), stop=(i == 7),
                        )
                    nc.vector.tensor_copy(vsb[:, tb, 0:64], pv[:, 0:64])
                    nc.vector.tensor_copy(vsb[:, tb, 65:129], pv[:, 64:128])

                yield 2150, (lambda: qk_item(wq_sb, qrot))
                yield 2150, (lambda: qk_item(wk_sb, krot))
                for j in range(4):
                    yield 620, (lambda j=j: v_item(j))

            def outproj_items(ch):
                b, ci = divmod(ch, NCI)
                for jb in range(4):
                    def item(jb=jb):
                        lhs = ctx[:, ch, ds(jb * 128, 128)]
                        osb = sb3.tile([128, 2 * CH], dt.bfloat16, tag="osb",
                                        name="osb")
                        for e in range(2):
                            op = psA.tile([128, CH], dt.float32, tag="a", name="ops")
                            nc.tensor.matmul(
                                op[:], lhs, wo_sb[:, ds(e * CH, CH)],
                                start=True, stop=True,
                            )
                            if e == 0:
                                nc.scalar.copy(osb[:, ds(0, CH)], op[:])
                            else:
                                nc.vector.tensor_copy(osb[:, ds(CH, CH)], op[:])
                        nc.sync.dma_start(
                            out_ap[b * 16 + ci * 4 + jb, :, :], osb[:]
                        )
                    yield 960, item

            def attn_emissions(ch):
                """Software-pipelined attention for chunk ch: yields
                (est_pe_ns, fn) for S(0), S(1), P(0), S(2), P(1), ...,
                P(last), tail."""
                b, ci = divmod(ch, NCI)
                nkb = 4 * ci + 4
                # one accumulator tile per head: exactly 1 PSUM bank each, so
                # each accumulation group owns its bank, and the pool (bufs=2)
                # lets head-h of chunk ch reuse head-h of chunk ch-1 the
                # moment that head's own tail mul has drained
                ctxph = [
                    cps.tile([65, CH], dt.float32, tag="ctx", name=f"ctxp{h}")
                    for h in range(2)
                ]
                prs = [None] * nkb

                def s_unit(kj):
                    diag = kj >= 4 * ci
                    off = 128 * (kj - 4 * ci) if diag else 0
                    n = CH - off
                    sp = sps.tile([128, 2 * CH], dt.float32, tag="s", name="sp")
                    for h in range(2):
                        hs = h * 64
                        k_ap = krot[hs:hs + 64, b * 4 + kj // 4,
                                    ds((kj % 4) * 128, 128)]
                        nc.tensor.matmul(
                            sp[:, ds(h * CH, n)], k_ap,
                            qrot[hs:hs + 64, ch, ds(off, n)],
                            start=True, stop=True,
                        )
                    pr = sb2.tile([128, 2 * CH], dt.bfloat16, tag="pr", name="pr")
                    if n == CH:
                        nc.scalar.activation(pr[:], sp[:], AF.Exp)
                    else:
                        sp3 = sp.rearrange("p (g q) -> p g q", g=2)
                        pr3 = pr.rearrange("p (g q) -> p g q", g=2)
                        nc.scalar.activation(
                            pr3[:, :, 0:n], sp3[:, :, 0:n], AF.Exp
                        )
                    if diag:
                        pr2 = pr.rearrange("p (g q) -> p g q", g=2)
                        m2 = mask_sb[:, 0:1, 0:128]
                        nc.vector.tensor_mul(
                            pr2[:, :, 0:128], pr2[:, :, 0:128],
                            m2.broadcast_to([128, 2, 128]),
                        )
                    prs[kj] = pr

                def p_unit(kj):
                    diag = kj >= 4 * ci
                    off = 128 * (kj - 4 * ci) if diag else 0
                    n = CH - off
                    pr = prs[kj]
                    for h in range(2):
                        nc.tensor.matmul(
                            ctxph[h][:, ds(off, n)],
                            vsb[:, b * 16 + kj, ds(h * 65, 65)],
                            pr[:, ds(h * CH, n)],
                            start=(kj == 0), stop=(kj == nkb - 1),
                        )
                    prs[kj] = None

                def tail_h(h):
                    den = sbt.tile([1, CH], dt.float32, tag="den", name="den")
                    nc.scalar.copy(den[:], ctxph[h][64:65, :])
                    rec = sbt.tile([1, CH], dt.float32, tag="rec", name="rec")
                    nc.vector.reciprocal_approx_fast(rec[:], den[:])
                    rbc = sbt.tile([64, CH], dt.float32, tag="rbc", name="rbc")
                    nc.gpsimd.partition_broadcast(rbc[:], rec[:])
                    nc.vector.tensor_mul(
                        ctx[h * 64:(h + 1) * 64, ch, :],
                        ctxph[h][0:64, :], rbc[:],
                    )

                def s_cost(kj):
                    # biased high so pacing drops fillers right after S,
                    # covering the exp latency before PV(kj-1)
                    diag = kj >= 4 * ci
                    n = CH - (128 * (kj - 4 * ci) if diag else 0)
                    return int(n * 0.43) + 460

                def p_cost(kj):
                    diag = kj >= 4 * ci
                    n = CH - (128 * (kj - 4 * ci) if diag else 0)
                    return 2 * (int(n * 0.43) + 25)

                # depth-2 software pipeline: S0 S1 S2 P0 S3 P1 ... so exp(kj)
                # has ~two scores-units of runway before P(kj) issues
                depth = min(2, nkb - 1)
                for kj in range(nkb):
                    yield s_cost(kj), (lambda kj=kj: s_unit(kj))
                    if kj >= depth:
                        yield p_cost(kj - depth), (lambda kj=kj: p_unit(kj - depth))
                for kj in range(nkb - depth, nkb):
                    yield p_cost(kj), (lambda kj=kj: p_unit(kj))
                yield 60, (lambda: tail_h(0))
                yield 60, (lambda: tail_h(1))

            # ---- prologue: projections for chunks 0 and 1 ----
            for _, fn in proj_items(0):
                fn()
            for _, fn in proj_items(1):
                fn()

            # ---- main loop: attention(ch) with fillers drawn from a global
            # carry-over reservoir. Pacing keeps cumulative filler cost
            # proportional to cumulative attention cost across the WHOLE run,
            # so leftovers flow into the late chunks (long attention phases,
            # no projection work left). Projection items carry a deadline one
            # chunk before their consumer so they always land with slack. ----
            def attn_costs(ch):
                b, ci = divmod(ch, NCI)
                nkb = 4 * ci + 4

                def n_of(kj):
                    return CH - (128 * (kj - 4 * ci) if kj >= 4 * ci else 0)

                costs = []
                depth = min(2, nkb - 1)
                for kj in range(nkb):
                    costs.append(int(n_of(kj) * 0.43) + 460)
                    if kj >= depth:
                        costs.append(2 * (int(n_of(kj - depth) * 0.43) + 25))
                for kj in range(nkb - depth, nkb):
                    costs.append(2 * (int(n_of(kj) * 0.43) + 25))
                costs.append(120)
                return costs

            A_total = sum(sum(attn_costs(c)) for c in range(NCHUNK))
            state = {"A": 0, "F": 0, "K": 0}
            reservoir = deque()  # (cost, fn, deadline_chunk)

            def pump_one():
                c_, f_, _ = reservoir.popleft()
                f_()
                state["F"] += c_

            for ch in range(NCHUNK):
                if ch + 3 < NCHUNK:
                    reservoir.append((50, (lambda c=ch + 3: xt_dma(c)), ch + 2))
                    state["K"] += 50
                if ch + 2 < NCHUNK:
                    for c_, f_ in proj_items(ch + 2):
                        reservoir.append((c_, f_, ch + 1))
                        state["K"] += c_
                if ch >= 1:
                    for c_, f_ in outproj_items(ch - 1):
                        reservoir.append((c_, f_, 1 << 30))
                        state["K"] += c_
                while any(dl <= ch for _, _, dl in reservoir):
                    pump_one()
                for c, fn in attn_emissions(ch):
                    fn()
                    state["A"] += c
                    while reservoir and state["F"] * A_total < state["K"] * state["A"]:
                        pump_one()
            while reservoir:
                pump_one()

            # ---- epilogue ----
            for _, fn in outproj_items(NCHUNK - 1):
                fn()

    nc.compile()
    return nc


def _get_program():
    global _PROGRAM
    if _PROGRAM is None:
        _PROGRAM = _build_program()
    return _PROGRAM


def _host_prep(x, w_qkv, w_out):
    """Build the per-core and shared device input arrays."""
    # half-split permutation of head_dim: (0,2,..,62, 1,3,..,63)
    perm = np.empty(HD, dtype=np.int64)
    perm[:32] = 2 * np.arange(32)
    perm[32:] = 2 * np.arange(32) + 1

    inv_freq = 1.0 / (10000.0 ** (np.arange(0, HD, 2, dtype=np.float64) / HD))
    ang = np.arange(S, dtype=np.float64)[None, :] * inv_freq[:, None]  # [32, S]
    cos64 = np.concatenate([np.cos(ang), np.cos(ang)], 0)  # [64, S] half-split
    sin64 = np.concatenate([np.sin(ang), np.sin(ang)], 0)
    cosT = np.ascontiguousarray(np.concatenate([cos64, cos64], 0)).astype(BF16)
    sinT = np.ascontiguousarray(np.concatenate([sin64, sin64], 0)).astype(BF16)

    # rotate_half in half-split coords as a matmul: out = R q
    Rh = np.zeros((HD, HD), dtype=np.float32)
    i32 = np.arange(32)
    Rh[i32, 32 + i32] = -1.0
    Rh[32 + i32, i32] = 1.0
    R = np.zeros((128, 128), dtype=np.float32)
    R[:64, :64] = Rh
    R[64:, 64:] = Rh
    rT = np.ascontiguousarray(R.T).astype(BF16)

    xT = np.ascontiguousarray(x.reshape(T, D).T).astype(BF16)

    ql = np.arange(CH)[None, :]
    kl = np.arange(128)[:, None]
    masks = np.concatenate(
        [(ql >= 128 * j + kl) for j in range(4)], axis=1
    ).astype(BF16)  # [128, 4*CH]

    def pack_w(wT):
        # [D, 128] -> [128, 8*128]: partition p holds contraction rows
        # {p, 128+p, ...} so the device DMA is fully contiguous per line
        return np.ascontiguousarray(
            wT.reshape(8, 128, 128).transpose(1, 0, 2).reshape(128, 8 * 128)
        ).astype(BF16)

    shared = dict(xT=xT, cosT=cosT, sinT=sinT, rT=rT, masks=masks)
    per_core = []
    for c in range(NCORES):
        h0 = 2 * c
        rows_q = np.concatenate([h * HD + perm for h in (h0, h0 + 1)])
        rows_k = np.concatenate([D + h * HD + perm for h in (h0, h0 + 1)])
        rows_v = np.concatenate(
            [2 * D + h * HD + np.arange(HD) for h in (h0, h0 + 1)]
        )
        per_core.append(
            dict(
                wqT=pack_w((w_qkv[rows_q, :] * 0.125).T.astype(np.float32)),
                wkT=pack_w(w_qkv[rows_k, :].T.astype(np.float32)),
                wvT=pack_w(w_qkv[rows_v, :].T.astype(np.float32)),
                woT=np.ascontiguousarray(w_out[:, c * 128:(c + 1) * 128].T).astype(BF16),
            )
        )
    return shared, per_core


def run(x, w_qkv, w_out, trace=False):
    """Run the sharded kernel; returns (out [B,S,D] f32, BassKernelResults)."""
    from concourse import bass_utils

    x = np.asarray(x, dtype=np.float32)
    w_qkv = np.asarray(w_qkv, dtype=np.float32)
    w_out = np.asarray(w_out, dtype=np.float32)

    shared, per_core = _host_prep(x, w_qkv, w_out)
    nc = _get_program()

    in_maps = [{**shared, **per_core[c]} for c in range(NCORES)]
    last_exc = None
    for _attempt in range(3):
        try:
            res = bass_utils.run_bass_kernel_spmd(
                nc, in_maps, core_ids=list(range(NCORES)), trace=trace
            )
            break
        except Exception as e:  # transient NRT/axon failures — retry
            last_exc = e
    else:
        raise last_exc
    out = res.results[0]["out"].astype(np.float32)
    for c in range(1, NCORES):
        out = out + res.results[c]["out"].astype(np.float32)
    return out.reshape(B, S, D), res


def kernel(x, w_qkv, w_out, src_mask=None, src_padding=None, is_causal=1):
    out, _ = run(x, w_qkv, w_out)
    return out
